# revision 6
# baseline (speedup 1.0000x reference)
"""Trainium2 Bass kernel for stacked-Linear dense MLP:
    out[1024, 32768] = x[1024, 512] @ W[32768, 512].T + b[32768]

Strategy: column-parallel over 8 NeuronCores. Core c owns W rows
[c*4096, (c+1)*4096) -> output columns of the same range; x replicated.
On-chip: bf16 matmul (fp32 PSUM accumulate), bias added on DVE during
PSUM->SBUF evacuation (cast to bf16), bf16 output upcast to fp32 on host.

Perf structure:
  - Host pre-arranges x/W into SBUF-image layouts (1-4 KiB contiguous
    per partition per DMA descriptor).
  - n-OUTER loop: each W chunk (512 KiB) feeds 8 m-tile matmul groups
    (~6.8us of PE work vs ~1.5-3us of load), PE runs dense once started.
  - W chunk DMAs are chained (each waits on the previous) so chunk 0
    completes ASAP instead of all chunks time-sharing bandwidth and
    completing together at the end.
  - x DMAs likewise split into 4 chained chunks on the other HWDGE ring.
  - bias DMA + gpsimd partition_broadcast issued first (hidden).
  - PE warmup matmuls un-throttle the HAM clock gate before real work.
  - Output DMAs (bf16, half the bytes of fp32) alternate across rings.
"""

import sys

sys.path.insert(0, "/opt/trn_rl_repo")

import numpy as np
import ml_dtypes

# ---- problem constants (hardcoded per contract) ----
B = 1024          # batch (matmul M)
K = 512           # hidden size (contraction)
N_TOTAL = 32768   # hidden_size * map_element_size
N_CORES = 8
NS = N_TOTAL // N_CORES  # 4096 output cols per core

KT = K // 128     # 4 k-tiles
MT = B // 128     # 8 m-tiles
NCH = NS // 512   # 8 n-chunks of 512 (one PSUM bank each)

OUT_BF16 = True   # device writes bf16, host upcasts to fp32

_CACHE = {}


def _build_program():
    import concourse.bacc as bacc
    import concourse.mybir as mybir
    from concourse.bass import ds, ts
    from concourse.tile import TileContext
    from concourse.tile_rust import add_dep_helper
    from contextlib import ExitStack

    nc = bacc.Bacc("TRN2", target_bir_lowering=False, debug=False)

    out_dt = mybir.dt.bfloat16 if OUT_BF16 else mybir.dt.float32

    # host-prepared SBUF-image layouts (see _prep_inputs)
    xh = nc.dram_tensor("xh", [128, MT, KT, 128], mybir.dt.bfloat16, kind="ExternalInput").ap()
    wh = nc.dram_tensor("wh", [128, NCH, KT, 512], mybir.dt.bfloat16, kind="ExternalInput").ap()
    bias = nc.dram_tensor("bias", [1, NS], mybir.dt.float32, kind="ExternalInput").ap()
    out = nc.dram_tensor("out", [B, NS], out_dt, kind="ExternalOutput").ap()

    with TileContext(nc) as tc:
        with ExitStack() as ctx:
            const = ctx.enter_context(tc.tile_pool(name="const", bufs=1))
            outp = ctx.enter_context(tc.tile_pool(name="outp", bufs=6))
            psum = ctx.enter_context(tc.tile_pool(name="psum", bufs=7, space="PSUM"))
            wpool = ctx.enter_context(tc.tile_pool(name="wpool", bufs=1))

            # --- bias first: tiny DMA + gpsimd broadcast, hidden under loads
            bias_sb = const.tile([128, NS], mybir.dt.float32, tag="bias")
            nc.scalar.dma_start(bias_sb[0:1, :], bias)
            nc.gpsimd.partition_broadcast(bias_sb[:], bias_sb[0:1, :])

            # --- PE warmup: un-throttle HAM while input DMAs run
            warm = const.tile([128, 512], mybir.dt.bfloat16, tag="warm")
            warm_ps = psum.tile([128, 512], mybir.dt.float32, tag="warmps", bufs=1)
            nc.vector.memset(warm[:], 0)
            for _ in range(10):
                nc.tensor.matmul(
                    warm_ps[:], lhsT=warm[:, 0:128], rhs=warm[:], start=True, stop=True
                )
            warm_sink = const.tile([128, 512], mybir.dt.float32, tag="warmsink")
            nc.vector.tensor_copy(warm_sink[:], warm_ps[:])  # keep warmups live

            # --- x: 4 chained chunks (2 m-tiles each) on the scalar ring
            xh_sb = const.tile([128, MT, KT, 128], mybir.dt.bfloat16, tag="xh")
            prev = None
            for c in range(4):
                dma = nc.scalar.dma_start(
                    xh_sb[:, ds(2 * c, 2)], xh[:, ds(2 * c, 2)]
                )
                if prev is not None:
                    add_dep_helper(dma.ins, prev.ins, reason="chain xh DMAs")
                prev = dma

            # --- W: 8 chained chunks on the sync ring, in consumption order
            wt_tiles = []
            prev = None
            for n in range(NCH):
                t = wpool.tile([128, KT, 512], mybir.dt.bfloat16, tag=f"wt{n}")
                dma = nc.sync.dma_start(t[:], wh[:, n])
                if prev is not None:
                    add_dep_helper(dma.ins, prev.ins, reason="chain W DMAs")
                prev = dma
                wt_tiles.append(t)

            # --- main loop: n-chunks outer so PE tracks W arrival
            for n in range(NCH):
                for m in range(MT):
                    ps = psum.tile([128, 512], mybir.dt.float32)
                    for k in range(KT):
                        nc.tensor.matmul(
                            ps[:],
                            lhsT=xh_sb[:, m, k, :],
                            rhs=wt_tiles[n][:, k, :],
                            start=(k == 0),
                            stop=(k == KT - 1),
                        )
                    ot = outp.tile([128, 512], out_dt)
                    nc.vector.tensor_add(ot[:], ps[:], bias_sb[:, ds(n * 512, 512)])
                    eng = nc.sync if (n * MT + m) % 2 == 0 else nc.scalar
                    eng.dma_start(out[ts(m, 128), ds(n * 512, 512)], ot[:])

    nc.compile()
    return nc


def _get_program():
    if "nc" not in _CACHE:
        _CACHE["nc"] = _build_program()
    return _CACHE["nc"]


def _prep_inputs(x, W, b):
    bf16 = ml_dtypes.bfloat16
    x = np.asarray(x, dtype=np.float32)
    W = np.asarray(W, dtype=np.float32)
    b = np.asarray(b, dtype=np.float32)
    # xh[p, mt, kt, m] = x[mt*128 + m, kt*128 + p]
    xh = np.ascontiguousarray(
        x.T.reshape(KT, 128, MT, 128).transpose(1, 2, 0, 3)
    ).astype(bf16)
    in_maps = []
    for c in range(N_CORES):
        sl = slice(c * NS, (c + 1) * NS)
        # wh[p, n, kt, j] = W[c*NS + n*512 + j, kt*128 + p]
        wh = np.ascontiguousarray(
            W[sl, :].T.reshape(KT, 128, NCH, 512).transpose(1, 2, 0, 3)
        ).astype(bf16)
        bc = np.ascontiguousarray(b[sl].reshape(1, NS))
        in_maps.append({"xh": xh, "wh": wh, "bias": bc})
    return in_maps


def _run(x, W, b, trace=False):
    from concourse.bass_utils import run_bass_kernel_spmd

    nc = _get_program()
    in_maps = _prep_inputs(x, W, b)
    res = run_bass_kernel_spmd(nc, in_maps, list(range(N_CORES)), trace=trace)
    _CACHE["last_result"] = res
    out = np.concatenate([r["out"] for r in res.results], axis=1)
    return out.astype(np.float32)


def kernel(x, W, b):
    return _run(x, W, b, trace=False)


def kernel_profiled(x, W, b):
    """Same as kernel() but with NTFF tracing; returns (out, BassKernelResults)."""
    out = _run(x, W, b, trace=True)
    return out, _CACHE["last_result"]


# revision 9
# speedup vs baseline: 1.0632x; 1.0632x over previous
"""Trainium2 Bass kernel for stacked-Linear dense MLP:
    out[1024, 32768] = x[1024, 512] @ W[32768, 512].T + b[32768]

Strategy: column-parallel over 8 NeuronCores. Core c owns W rows
[c*4096, (c+1)*4096) -> output columns of the same range; x replicated.
On-chip: bf16 matmul (fp32 PSUM accumulate), bias added on DVE during
PSUM->SBUF evacuation (cast to bf16), bf16 output upcast to fp32 on host.

Perf structure:
  - Host pre-arranges x/W into SBUF-image layouts (1-4 KiB contiguous
    per partition per DMA descriptor).
  - n-OUTER loop: each W chunk (512 KiB) feeds 8 m-tile matmul groups
    (~6.8us of PE work vs ~1.5-3us of load), PE runs dense once started.
  - W chunk DMAs are chained (each waits on the previous) so chunk 0
    completes ASAP instead of all chunks time-sharing bandwidth and
    completing together at the end.
  - x DMAs likewise split into 4 chained chunks on the other HWDGE ring.
  - bias DMA + gpsimd partition_broadcast issued first (hidden).
  - PE warmup matmuls un-throttle the HAM clock gate before real work.
  - Output DMAs (bf16, half the bytes of fp32) alternate across rings.
"""

import sys

sys.path.insert(0, "/opt/trn_rl_repo")

import numpy as np
import ml_dtypes

# ---- problem constants (hardcoded per contract) ----
B = 1024          # batch (matmul M)
K = 512           # hidden size (contraction)
N_TOTAL = 32768   # hidden_size * map_element_size
N_CORES = 8
NS = N_TOTAL // N_CORES  # 4096 output cols per core

KT = K // 128     # 4 k-tiles
MT = B // 128     # 8 m-tiles
NCH = NS // 512   # 8 n-chunks of 512 (one PSUM bank each)

OUT_BF16 = True   # device writes bf16, host upcasts to fp32

_CACHE = {}


def _build_program():
    import concourse.bacc as bacc
    import concourse.mybir as mybir
    from concourse.bass import ds, ts
    from concourse.tile import TileContext
    from concourse.tile_rust import add_dep_helper
    from contextlib import ExitStack

    nc = bacc.Bacc("TRN2", target_bir_lowering=False, debug=False)

    out_dt = mybir.dt.bfloat16 if OUT_BF16 else mybir.dt.float32

    # host-prepared SBUF-image layouts (see _prep_inputs)
    xh = nc.dram_tensor("xh", [128, MT, KT, 128], mybir.dt.bfloat16, kind="ExternalInput").ap()
    wh = nc.dram_tensor("wh", [128, NCH, KT, 512], mybir.dt.bfloat16, kind="ExternalInput").ap()
    bias = nc.dram_tensor("bias", [1, NS], mybir.dt.float32, kind="ExternalInput").ap()
    out = nc.dram_tensor("out", [B, NS], out_dt, kind="ExternalOutput").ap()

    with TileContext(nc) as tc:
        with ExitStack() as ctx:
            const = ctx.enter_context(tc.tile_pool(name="const", bufs=1))
            outp = ctx.enter_context(tc.tile_pool(name="outp", bufs=12))
            psum = ctx.enter_context(tc.tile_pool(name="psum", bufs=7, space="PSUM"))
            wpool = ctx.enter_context(tc.tile_pool(name="wpool", bufs=1))

            # --- bias first: tiny DMA + gpsimd broadcast, hidden under loads
            bias_sb = const.tile([128, NS], mybir.dt.float32, tag="bias")
            nc.scalar.dma_start(bias_sb[0:1, :], bias)
            nc.gpsimd.partition_broadcast(bias_sb[:], bias_sb[0:1, :])

            # --- PE warmup: un-throttle HAM while input DMAs run
            warm = const.tile([128, 512], mybir.dt.bfloat16, tag="warm")
            warm_ps = psum.tile([128, 512], mybir.dt.float32, tag="warmps", bufs=1)
            nc.vector.memset(warm[:], 0)
            for _ in range(10):
                nc.tensor.matmul(
                    warm_ps[:], lhsT=warm[:, 0:128], rhs=warm[:], start=True, stop=True
                )
            warm_sink = const.tile([128, 512], mybir.dt.float32, tag="warmsink")
            nc.vector.tensor_copy(warm_sink[:], warm_ps[:])  # keep warmups live

            # --- x: 2 chained chunks (m0-1 first: the early-critical bytes)
            xh_sb = const.tile([128, MT, KT, 128], mybir.dt.bfloat16, tag="xh")
            dma_x0 = nc.scalar.dma_start(xh_sb[:, ds(0, 2)], xh[:, ds(0, 2)])
            dma_x1 = nc.scalar.dma_start(xh_sb[:, ds(2, 6)], xh[:, ds(2, 6)])
            add_dep_helper(dma_x1.ins, dma_x0.ins, reason="chain xh DMAs")

            # --- W: 4 chained 1-MiB chunks (2 n-chunks each) on the sync ring.
            # Strict chain => chunk 0 completes ASAP (concurrent DMAs share
            # bandwidth and all complete together); each link feeds ~13.6us
            # of PE work, far above its ~4us load+receipt time.
            wt_tiles = []
            prev = None
            for c in range(NCH // 2):
                t = wpool.tile([128, 2, KT, 512], mybir.dt.bfloat16, tag=f"wt{c}")
                dma = nc.sync.dma_start(t[:], wh[:, ds(2 * c, 2)])
                if prev is not None:
                    add_dep_helper(dma.ins, prev.ins, reason="chain W DMAs")
                prev = dma
                wt_tiles.append(t)

            # --- main loop: n-chunks outer so PE tracks W arrival
            for n in range(NCH):
                for m in range(MT):
                    ps = psum.tile([128, 512], mybir.dt.float32)
                    for k in range(KT):
                        nc.tensor.matmul(
                            ps[:],
                            lhsT=xh_sb[:, m, k, :],
                            rhs=wt_tiles[n // 2][:, n % 2, k, :],
                            start=(k == 0),
                            stop=(k == KT - 1),
                        )
                    ot = outp.tile([128, 512], out_dt)
                    nc.vector.tensor_add(ot[:], ps[:], bias_sb[:, ds(n * 512, 512)])
                    eng = nc.sync if (n * MT + m) % 2 == 0 else nc.scalar
                    eng.dma_start(out[ts(m, 128), ds(n * 512, 512)], ot[:])

    nc.compile()
    return nc


def _get_program():
    if "nc" not in _CACHE:
        _CACHE["nc"] = _build_program()
    return _CACHE["nc"]


def _prep_inputs(x, W, b):
    bf16 = ml_dtypes.bfloat16
    x = np.asarray(x, dtype=np.float32)
    W = np.asarray(W, dtype=np.float32)
    b = np.asarray(b, dtype=np.float32)
    # xh[p, mt, kt, m] = x[mt*128 + m, kt*128 + p]
    xh = np.ascontiguousarray(
        x.T.reshape(KT, 128, MT, 128).transpose(1, 2, 0, 3)
    ).astype(bf16)
    in_maps = []
    for c in range(N_CORES):
        sl = slice(c * NS, (c + 1) * NS)
        # wh[p, n, kt, j] = W[c*NS + n*512 + j, kt*128 + p]
        wh = np.ascontiguousarray(
            W[sl, :].T.reshape(KT, 128, NCH, 512).transpose(1, 2, 0, 3)
        ).astype(bf16)
        bc = np.ascontiguousarray(b[sl].reshape(1, NS))
        in_maps.append({"xh": xh, "wh": wh, "bias": bc})
    return in_maps


def _run(x, W, b, trace=False):
    from concourse.bass_utils import run_bass_kernel_spmd

    nc = _get_program()
    in_maps = _prep_inputs(x, W, b)
    res = run_bass_kernel_spmd(nc, in_maps, list(range(N_CORES)), trace=trace)
    _CACHE["last_result"] = res
    out = np.concatenate([r["out"] for r in res.results], axis=1)
    return out.astype(np.float32)


def kernel(x, W, b):
    return _run(x, W, b, trace=False)


def kernel_profiled(x, W, b):
    """Same as kernel() but with NTFF tracing; returns (out, BassKernelResults)."""
    out = _run(x, W, b, trace=True)
    return out, _CACHE["last_result"]


# revision 10
# speedup vs baseline: 1.1269x; 1.0599x over previous
"""Trainium2 Bass kernel for stacked-Linear dense MLP:
    out[1024, 32768] = x[1024, 512] @ W[32768, 512].T + b[32768]

Strategy: column-parallel over 8 NeuronCores. Core c owns W rows
[c*4096, (c+1)*4096) -> output columns of the same range; x replicated.
On-chip: bf16 matmul (fp32 PSUM accumulate), bias added on DVE during
PSUM->SBUF evacuation (cast to bf16), bf16 output upcast to fp32 on host.

Perf structure:
  - Host pre-arranges x/W into SBUF-image layouts (1-4 KiB contiguous
    per partition per DMA descriptor).
  - n-OUTER loop: each W chunk (512 KiB) feeds 8 m-tile matmul groups
    (~6.8us of PE work vs ~1.5-3us of load), PE runs dense once started.
  - W chunk DMAs are chained (each waits on the previous) so chunk 0
    completes ASAP instead of all chunks time-sharing bandwidth and
    completing together at the end.
  - x DMAs likewise split into 4 chained chunks on the other HWDGE ring.
  - bias DMA + gpsimd partition_broadcast issued first (hidden).
  - PE warmup matmuls un-throttle the HAM clock gate before real work.
  - Output DMAs (bf16, half the bytes of fp32) alternate across rings.
"""

import sys

sys.path.insert(0, "/opt/trn_rl_repo")

import numpy as np
import ml_dtypes

# ---- problem constants (hardcoded per contract) ----
B = 1024          # batch (matmul M)
K = 512           # hidden size (contraction)
N_TOTAL = 32768   # hidden_size * map_element_size
N_CORES = 8
NS = N_TOTAL // N_CORES  # 4096 output cols per core

KT = K // 128     # 4 k-tiles
MT = B // 128     # 8 m-tiles
NCH = NS // 512   # 8 n-chunks of 512 (one PSUM bank each)

OUT_BF16 = True   # device writes bf16, host upcasts to fp32

_CACHE = {}


def _build_program():
    import concourse.bacc as bacc
    import concourse.mybir as mybir
    from concourse.bass import ds, ts
    from concourse.tile import TileContext
    from concourse.tile_rust import add_dep_helper
    from contextlib import ExitStack

    nc = bacc.Bacc("TRN2", target_bir_lowering=False, debug=False)

    out_dt = mybir.dt.bfloat16 if OUT_BF16 else mybir.dt.float32

    # host-prepared SBUF-image layouts (see _prep_inputs)
    xh = nc.dram_tensor("xh", [128, MT, KT, 128], mybir.dt.bfloat16, kind="ExternalInput").ap()
    wh = nc.dram_tensor("wh", [128, NCH, KT, 512], mybir.dt.bfloat16, kind="ExternalInput").ap()
    bias = nc.dram_tensor("bias", [1, NS], mybir.dt.float32, kind="ExternalInput").ap()
    out = nc.dram_tensor("out", [B, NS], out_dt, kind="ExternalOutput").ap()

    with TileContext(nc) as tc:
        with ExitStack() as ctx:
            const = ctx.enter_context(tc.tile_pool(name="const", bufs=1))
            outp = ctx.enter_context(tc.tile_pool(name="outp", bufs=12))
            psum = ctx.enter_context(tc.tile_pool(name="psum", bufs=7, space="PSUM"))
            wpool = ctx.enter_context(tc.tile_pool(name="wpool", bufs=1))

            # --- bias first: tiny DMA + per-chunk gpsimd broadcasts (pipelined)
            bias_sb = const.tile([128, NS], mybir.dt.float32, tag="bias")
            bias_dma = nc.scalar.dma_start(bias_sb[0:1, :], bias)
            for n in range(NCH):
                nc.gpsimd.partition_broadcast(
                    bias_sb[:, ds(n * 512, 512)], bias_sb[0:1, ds(n * 512, 512)]
                )

            # --- PE warmup: un-throttle HAM while input DMAs run
            warm = const.tile([128, 512], mybir.dt.bfloat16, tag="warm")
            warm_ps = psum.tile([128, 512], mybir.dt.float32, tag="warmps", bufs=1)
            nc.vector.memset(warm[:], 0)
            for _ in range(7):
                nc.tensor.matmul(
                    warm_ps[:], lhsT=warm[:, 0:128], rhs=warm[:], start=True, stop=True
                )
            warm_sink = const.tile([128, 512], mybir.dt.float32, tag="warmsink")
            nc.vector.tensor_copy(warm_sink[:], warm_ps[:])  # keep warmups live

            # --- x on the scalar ring: 2 chained chunks, early m-tiles first
            xh_sb = const.tile([128, MT, KT, 128], mybir.dt.bfloat16, tag="xh")
            dma_x0 = nc.scalar.dma_start(xh_sb[:, ds(0, 4)], xh[:, ds(0, 4)])
            dma_x1 = nc.scalar.dma_start(xh_sb[:, ds(4, 4)], xh[:, ds(4, 4)])
            add_dep_helper(dma_x1.ins, dma_x0.ins, reason="chain xh DMAs")

            # --- W on the sync ring: chained chunks of [1,2,2,3] n-chunks.
            # First link small (lands with xh chunk 0, unblocks the PE);
            # later links big (amortize per-DMA cost, still far ahead of
            # the PE's 6.8us-per-n-chunk consumption rate).
            W_SPLIT = [1, 2, 2, 3]
            wt_tiles = []
            n2cl = {}
            prev = None
            n0 = 0
            for c, sz in enumerate(W_SPLIT):
                t = wpool.tile([128, sz, KT, 512], mybir.dt.bfloat16, tag=f"wt{c}")
                dma = nc.sync.dma_start(t[:], wh[:, ds(n0, sz)])
                if prev is not None:
                    add_dep_helper(dma.ins, prev.ins, reason="chain W DMAs")
                prev = dma
                wt_tiles.append(t)
                for i in range(sz):
                    n2cl[n0 + i] = (c, i)
                n0 += sz

            # --- main loop: n-chunks outer so PE tracks W arrival
            for n in range(NCH):
                for m in range(MT):
                    g = n * MT + m
                    c, ln = n2cl[n]
                    ps = psum.tile([128, 512], mybir.dt.float32)
                    for k in range(KT):
                        nc.tensor.matmul(
                            ps[:],
                            lhsT=xh_sb[:, m, k, :],
                            rhs=wt_tiles[c][:, ln, k, :],
                            start=(k == 0),
                            stop=(k == KT - 1),
                        )
                    ot = outp.tile([128, 512], out_dt)
                    nc.vector.tensor_add(ot[:], ps[:], bias_sb[:, ds(n * 512, 512)])
                    # keep the sync ring clear for the W chain early on
                    if g < 20:
                        eng = nc.scalar
                    else:
                        eng = nc.sync if g % 2 == 0 else nc.scalar
                    eng.dma_start(out[ts(m, 128), ds(n * 512, 512)], ot[:])

    nc.compile()
    return nc


def _get_program():
    if "nc" not in _CACHE:
        _CACHE["nc"] = _build_program()
    return _CACHE["nc"]


def _prep_inputs(x, W, b):
    bf16 = ml_dtypes.bfloat16
    x = np.asarray(x, dtype=np.float32)
    W = np.asarray(W, dtype=np.float32)
    b = np.asarray(b, dtype=np.float32)
    # xh[p, mt, kt, m] = x[mt*128 + m, kt*128 + p]
    xh = np.ascontiguousarray(
        x.T.reshape(KT, 128, MT, 128).transpose(1, 2, 0, 3)
    ).astype(bf16)
    in_maps = []
    for c in range(N_CORES):
        sl = slice(c * NS, (c + 1) * NS)
        # wh[p, n, kt, j] = W[c*NS + n*512 + j, kt*128 + p]
        wh = np.ascontiguousarray(
            W[sl, :].T.reshape(KT, 128, NCH, 512).transpose(1, 2, 0, 3)
        ).astype(bf16)
        bc = np.ascontiguousarray(b[sl].reshape(1, NS))
        in_maps.append({"xh": xh, "wh": wh, "bias": bc})
    return in_maps


def _run(x, W, b, trace=False):
    from concourse.bass_utils import run_bass_kernel_spmd

    nc = _get_program()
    in_maps = _prep_inputs(x, W, b)
    res = run_bass_kernel_spmd(nc, in_maps, list(range(N_CORES)), trace=trace)
    _CACHE["last_result"] = res
    out = np.concatenate([r["out"] for r in res.results], axis=1)
    return out.astype(np.float32)


def kernel(x, W, b):
    return _run(x, W, b, trace=False)


def kernel_profiled(x, W, b):
    """Same as kernel() but with NTFF tracing; returns (out, BassKernelResults)."""
    out = _run(x, W, b, trace=True)
    return out, _CACHE["last_result"]
